# revision 1
# baseline (speedup 1.0000x reference)
"""EdgeDecoder Trainium2 kernel.

Math (per edge e with endpoints src, dst):
    x  = z[src] * z[dst]            # [128]
    x2 = z2[src] * z2[dst]          # [128]
    h  = relu(W1 @ concat(x, x2) + b1)   # [256]
    out = sigmoid(W2 @ h + b2)           # scalar

Strategy:
  - Host packs z||z2 into one fp16 table with 512B rows; each edge endpoint
    is fetched with ONE dma_gather index (flat mode -> contiguous 512B row
    lands in one partition: edge e -> partition e%128, chunk e//128).
  - int16 gather indices can't address 100k rows, so edges are grouped by
    (src%4, dst%4) residue classes; each gather call reads the table with a
    2048B row stride and a 512B residue offset, so idx = node//4 < 25000.
  - The 16 residue classes are split evenly across the 8 cores.
  - Gathers run on all 4 SWDGE queues (queue aligned to the scheduler's
    DMASW completion lane) - the single-queue descriptor ring depth caps
    throughput at ~60 GB/s, 4 queues reach ~140 GB/s. Transpose-mode
    gathers corrupt data when concurrent across queues, so gathers are
    flat (edge-major) and the feature-major layout for the matmuls is
    produced by xbar DMA transposes on the (otherwise idle) HWDGE path:
    x = zs*zd on DVE edge-major, then [128,128] transpose chunks.
  - Per 512-edge subtile: 4 fp16 matmuls (K=128) accumulate the [256,512]
    hidden pre-activations in PSUM, ACT applies bias+relu to fp16, 2 more
    matmuls reduce against W2, ACT applies bias+sigmoid.
"""

import numpy as np

N_CORES = 8
D = 128          # per-table feature dim
H = 256          # hidden dim
ROW_ELEMS = 2 * D          # one packed row: z feats ++ z2 feats (fp16)
GROUP = 4                  # residue grouping factor (int16 idx = node // 4)

_prog_cache = {}


def _round_up(x, m):
    return (x + m - 1) // m * m


def _build_program(cell, n_rows_over_g):
    """Build the SPMD bass program for per-cell gather+MLP. cell = padded
    edges per residue cell (multiple of 128)."""
    import concourse.bacc as bacc
    import concourse.mybir as mybir
    import concourse.tile as tile

    f16 = mybir.dt.float16
    f32 = mybir.dt.float32
    edge_n = 16 * cell
    nchunk = cell // 128
    nsub_full, rem = divmod(cell, 512)
    subs = [512] * nsub_full + ([rem] if rem else [])

    nc = bacc.Bacc("TRN2", target_bir_lowering=False, debug=False,
                   num_swdge_queues=4)
    zz = nc.dram_tensor("zz", [n_rows_over_g, GROUP * ROW_ELEMS], f16,
                        kind="ExternalInput")
    sidx = nc.dram_tensor("sidx", [128, edge_n // 16], mybir.dt.int16,
                          kind="ExternalInput")
    didx = nc.dram_tensor("didx", [128, edge_n // 16], mybir.dt.int16,
                          kind="ExternalInput")
    w1t = nc.dram_tensor("w1t", [ROW_ELEMS, H], f16, kind="ExternalInput")
    b1 = nc.dram_tensor("b1", [H, 1], f32, kind="ExternalInput")
    w2t = nc.dram_tensor("w2t", [H, 1], f16, kind="ExternalInput")
    b2 = nc.dram_tensor("b2", [1, 1], f32, kind="ExternalInput")
    out = nc.dram_tensor("out", [16, cell], f32, kind="ExternalOutput")

    with tile.TileContext(nc) as tc:
        with (
            tc.tile_pool(name="const", bufs=1) as cpool,
            tc.tile_pool(name="gath", bufs=2) as gpool,
            tc.tile_pool(name="xt", bufs=2) as xpool,
            tc.tile_pool(name="hbuf", bufs=3) as hpool,
            tc.tile_pool(name="obuf", bufs=2) as opool,
            tc.tile_pool(name="ps_h", bufs=2, space="PSUM") as pph,
            tc.tile_pool(name="ps_a", bufs=2, space="PSUM") as ppa,
        ):
            # Constants
            w1_t = [[cpool.tile([128, 128], f16, tag=f"w1_{kc}_{hc}", name=f"w1_{kc}_{hc}")
                     for hc in range(2)] for kc in range(2)]
            for kc in range(2):
                for hc in range(2):
                    nc.sync.dma_start(
                        out=w1_t[kc][hc][:],
                        in_=w1t[kc * 128:(kc + 1) * 128, hc * 128:(hc + 1) * 128])
            b1_t = [cpool.tile([128, 1], f32, tag=f"b1_{hc}", name=f"b1_{hc}") for hc in range(2)]
            for hc in range(2):
                nc.sync.dma_start(out=b1_t[hc][:], in_=b1[hc * 128:(hc + 1) * 128, :])
            w2_t = [cpool.tile([128, 1], f16, tag=f"w2_{hc}", name=f"w2_{hc}") for hc in range(2)]
            for hc in range(2):
                nc.sync.dma_start(out=w2_t[hc][:], in_=w2t[hc * 128:(hc + 1) * 128, :])
            b2_t = cpool.tile([1, 1], f32, tag="b2")
            nc.sync.dma_start(out=b2_t[:], in_=b2[:])
            sidx_t = cpool.tile([128, edge_n // 16], mybir.dt.int16, tag="sidx")
            nc.sync.dma_start(out=sidx_t[:], in_=sidx[:])
            didx_t = cpool.tile([128, edge_n // 16], mybir.dt.int16, tag="didx")
            nc.sync.dma_start(out=didx_t[:], in_=didx[:])

            wcol = cell // 16
            for c in range(16):
                r, s = c // 4, c % 4
                zs_t = gpool.tile([128, nchunk, ROW_ELEMS], f16, tag="zs")
                nc.gpsimd.dma_gather(
                    zs_t[:], zz[:, r * ROW_ELEMS:(r + 1) * ROW_ELEMS],
                    sidx_t[:, c * wcol:(c + 1) * wcol],
                    cell, cell, ROW_ELEMS,
                    elem_step=GROUP * ROW_ELEMS, transpose=False,
                    single_packet=False)
                zd_t = gpool.tile([128, nchunk, ROW_ELEMS], f16, tag="zd")
                nc.gpsimd.dma_gather(
                    zd_t[:], zz[:, s * ROW_ELEMS:(s + 1) * ROW_ELEMS],
                    didx_t[:, c * wcol:(c + 1) * wcol],
                    cell, cell, ROW_ELEMS,
                    elem_step=GROUP * ROW_ELEMS, transpose=False,
                    single_packet=False)
                # x = zs*zd edge-major, written to xr as [p, hc, j, f] so one
                # xbar call per cell yields feature-major planes.
                xr_t = xpool.tile([128, 2, nchunk, 128], f16, tag="xr")
                nc.vector.tensor_mul(
                    out=xr_t[:].rearrange("p h j f -> p j h f"),
                    in0=zs_t[:].rearrange("p j (h f) -> p j h f", h=2),
                    in1=zd_t[:].rearrange("p j (h f) -> p j h f", h=2))
                # xt[f, hc, j, l] = x[edge 128j+l, feat 128hc+f]
                xt_t = xpool.tile([128, 2, nchunk, 128], f16, tag="xt")
                nc.sync.dma_start(
                    out=xt_t[:].rearrange("p h j f -> p (h j) f"),
                    in_=xr_t[:].rearrange("p h j f -> p (h j f)"),
                    transpose=True)

                out_sb = opool.tile([1, cell], f32, tag="out")
                col = 0
                for si, w in enumerate(subs):
                    st = slice(col, col + w)
                    j0, j1 = 4 * si, 4 * si + w // 128
                    hs = []
                    for hc in range(2):
                        hp = pph.tile([128, 512], f32, tag=f"h{hc}", space="PSUM")
                        nc.tensor.matmul(
                            hp[:, :w], lhsT=w1_t[0][hc][:],
                            rhs=xt_t[:, 0, j0:j1, :].rearrange("p j f -> p (j f)"),
                            start=True, stop=False)
                        nc.tensor.matmul(
                            hp[:, :w], lhsT=w1_t[1][hc][:],
                            rhs=xt_t[:, 1, j0:j1, :].rearrange("p j f -> p (j f)"),
                            start=False, stop=True)
                        hsb = hpool.tile([128, 512], f16, tag=f"hs{hc}")
                        nc.scalar.activation(
                            hsb[:, :w], hp[:, :w],
                            mybir.ActivationFunctionType.Relu, bias=b1_t[hc][:])
                        hs.append(hsb)
                    ap = ppa.tile([1, 512], f32, tag="a", space="PSUM")
                    nc.tensor.matmul(ap[:, :w], lhsT=w2_t[0][:], rhs=hs[0][:, :w],
                                     start=True, stop=False)
                    nc.tensor.matmul(ap[:, :w], lhsT=w2_t[1][:], rhs=hs[1][:, :w],
                                     start=False, stop=True)
                    nc.scalar.activation(
                        out_sb[:, st], ap[:, :w],
                        mybir.ActivationFunctionType.Sigmoid, bias=b2_t[:])
                    col += w
                nc.sync.dma_start(out=out[c:c + 1, :], in_=out_sb[:])

    # Align each gather's SWDGE queue with its scheduler-assigned DMASW
    # completion lane (queue = lane % 4). Completions within one lane then
    # come from a single queue, preserving the in-order counting-semaphore
    # contract that the lane rotation assumes.
    from concourse.tile_scheduler import PROC_NAME_TO_IDX
    dmasw0 = PROC_NAME_TO_IDX["DMASW0"]
    for blk in nc.m.functions[0].blocks:
        for inst in blk.instructions:
            if isinstance(inst, mybir.InstDMAGatherAnt):
                lane = inst.bass_scheduled_proc - dmasw0
                assert 0 <= lane < 8, lane
                inst.queue_num = lane % 4
    nc.compile()
    return nc


def kernel(z, z2, edge, W1, b1, W2, b2):
    from concourse.bass_utils import run_bass_kernel_spmd

    n_nodes, d = z.shape
    e_total = edge.shape[1]
    assert d == D

    src = np.asarray(edge[0], dtype=np.int64)
    dst = np.asarray(edge[1], dtype=np.int64)

    # Packed fp16 table: row v = z[v] ++ z2[v]; grouped GROUP rows per
    # "super-row" so int16 indices (node // GROUP) stay in range.
    n_pad = _round_up(n_nodes, GROUP)
    zz = np.zeros((n_pad, ROW_ELEMS), dtype=np.float16)
    zz[:n_nodes, :D] = z.astype(np.float16)
    zz[:n_nodes, D:] = z2.astype(np.float16)
    zz_g = zz.reshape(n_pad // GROUP, GROUP * ROW_ELEMS)

    # Partition edges into 16 residue classes, split each class evenly
    # across the 8 cores.
    cls = (src % GROUP) * 4 + (dst % GROUP)
    order = np.argsort(cls, kind="stable")
    bounds = np.searchsorted(cls[order], np.arange(17))
    splits = {}   # (core, cls) -> global edge ids
    max_cnt = 0
    for ci in range(16):
        ids = order[bounds[ci]:bounds[ci + 1]]
        parts = np.array_split(ids, N_CORES)
        for core in range(N_CORES):
            splits[(core, ci)] = parts[core]
            max_cnt = max(max_cnt, len(parts[core]))
    cell = max(_round_up(max_cnt, 128), 512)
    edge_n = 16 * cell

    in_maps = []
    perms = []
    for core in range(N_CORES):
        s_loc = np.zeros(edge_n, dtype=np.int64)
        d_loc = np.zeros(edge_n, dtype=np.int64)
        perm = np.full(edge_n, -1, dtype=np.int64)
        for ci in range(16):
            ids = splits[(core, ci)]
            o = ci * cell
            n = len(ids)
            perm[o:o + n] = ids
            s_loc[o:o + n] = src[ids] // GROUP
            d_loc[o:o + n] = dst[ids] // GROUP
            # padding keeps idx 0 with matching residue (valid row)
        s16 = s_loc.astype(np.int16)
        d16 = d_loc.astype(np.int16)
        # wrap per cell-call: [16, cell//16] blocks, replicated to 128 parts
        sw = np.concatenate(
            [s16[c * cell:(c + 1) * cell].reshape(cell // 16, 16).T
             for c in range(16)], axis=1)
        dw = np.concatenate(
            [d16[c * cell:(c + 1) * cell].reshape(cell // 16, 16).T
             for c in range(16)], axis=1)
        in_maps.append({
            "zz": zz_g,
            "sidx": np.tile(sw, (8, 1)),
            "didx": np.tile(dw, (8, 1)),
            "w1t": np.ascontiguousarray(W1.T).astype(np.float16),
            "b1": np.asarray(b1, dtype=np.float32).reshape(H, 1),
            "w2t": np.ascontiguousarray(np.asarray(W2).T).astype(np.float16),
            "b2": np.asarray(b2, dtype=np.float32).reshape(1, 1),
        })
        perms.append(perm)

    key = (cell, zz_g.shape[0])
    if key not in _prog_cache:
        _prog_cache[key] = _build_program(cell, zz_g.shape[0])
    nc = _prog_cache[key]

    res = run_bass_kernel_spmd(nc, in_maps, core_ids=list(range(N_CORES)))

    out_full = np.empty((e_total,), dtype=np.float32)
    for core in range(N_CORES):
        vals = np.asarray(res.results[core]["out"], dtype=np.float32).ravel()
        p = perms[core]
        m = p >= 0
        out_full[p[m]] = vals[m]
    return out_full.reshape(e_total, 1)



# revision 3
# speedup vs baseline: 1.2484x; 1.2484x over previous
"""EdgeDecoder Trainium2 kernel, v2.

Same residue-class architecture as v1, but gathers are issued in 1024-index
chunks spread round-robin over all 4 SWDGE queues with an 8-deep buffer
pool, so the Pool engine streams descriptors continuously (~2.6 ns/idx
descriptor-generation floor) and all queues drain concurrently, instead of
one 4096-idx gather blocking the engine ~29 us on ring-full stalls.

Per 512-edge subtile: 4 fp16 matmuls (K=128) accumulate [256,512] hidden
pre-activations in PSUM, ACT applies bias+relu to fp16, 2 matmuls reduce
against W2, ACT applies bias+sigmoid.
"""

import numpy as np

N_CORES = 8
D = 128
H = 256
ROW_ELEMS = 2 * D
GROUP = 4
CHUNK = 1024             # edges per gather instruction

_prog_cache = {}


def _round_up(x, m):
    return (x + m - 1) // m * m


def _build_program(cell, n_rows_over_g):
    import concourse.bacc as bacc
    import concourse.mybir as mybir
    import concourse.tile as tile

    f16 = mybir.dt.float16
    f32 = mybir.dt.float32
    edge_n = 16 * cell
    nchunk = cell // 128          # 128-edge groups per cell
    gch = cell // CHUNK           # gather chunks per cell
    nsub_full, rem = divmod(cell, 512)
    subs = [512] * nsub_full + ([rem] if rem else [])

    nc = bacc.Bacc("TRN2", target_bir_lowering=False, debug=False,
                   num_swdge_queues=4)
    zz = nc.dram_tensor("zz", [n_rows_over_g, GROUP * ROW_ELEMS], f16,
                        kind="ExternalInput")
    sidx = nc.dram_tensor("sidx", [128, edge_n // 16], mybir.dt.int16,
                          kind="ExternalInput")
    didx = nc.dram_tensor("didx", [128, edge_n // 16], mybir.dt.int16,
                          kind="ExternalInput")
    w1t = nc.dram_tensor("w1t", [ROW_ELEMS, H], f16, kind="ExternalInput")
    b1 = nc.dram_tensor("b1", [H, 1], f32, kind="ExternalInput")
    w2t = nc.dram_tensor("w2t", [H, 1], f16, kind="ExternalInput")
    b2 = nc.dram_tensor("b2", [1, 1], f32, kind="ExternalInput")
    out = nc.dram_tensor("out", [16, cell], f32, kind="ExternalOutput")

    with tile.TileContext(nc) as tc:
        with (
            tc.tile_pool(name="const", bufs=1) as cpool,
            tc.tile_pool(name="gath", bufs=10) as gpool,
            tc.tile_pool(name="xr", bufs=2) as xrpool,
            tc.tile_pool(name="xt", bufs=2) as xtpool,
            tc.tile_pool(name="hbuf", bufs=3) as hpool,
            tc.tile_pool(name="obuf", bufs=2) as opool,
            tc.tile_pool(name="ps_h", bufs=2, space="PSUM") as pph,
            tc.tile_pool(name="ps_a", bufs=2, space="PSUM") as ppa,
        ):
            w1_t = [[cpool.tile([128, 128], f16, tag=f"w1_{kc}_{hc}",
                                name=f"w1_{kc}_{hc}")
                     for hc in range(2)] for kc in range(2)]
            for kc in range(2):
                for hc in range(2):
                    nc.sync.dma_start(
                        out=w1_t[kc][hc][:],
                        in_=w1t[kc * 128:(kc + 1) * 128,
                                hc * 128:(hc + 1) * 128])
            b1_t = [cpool.tile([128, 1], f32, tag=f"b1_{hc}", name=f"b1_{hc}")
                    for hc in range(2)]
            for hc in range(2):
                nc.sync.dma_start(out=b1_t[hc][:],
                                  in_=b1[hc * 128:(hc + 1) * 128, :])
            w2_t = [cpool.tile([128, 1], f16, tag=f"w2_{hc}", name=f"w2_{hc}")
                    for hc in range(2)]
            for hc in range(2):
                nc.sync.dma_start(out=w2_t[hc][:],
                                  in_=w2t[hc * 128:(hc + 1) * 128, :])
            b2_t = cpool.tile([1, 1], f32, tag="b2")
            nc.sync.dma_start(out=b2_t[:], in_=b2[:])
            # Split the idx preload: cell 0's slice first so its gathers
            # start ~10us earlier (subtile deps release them before the
            # remaining 4MB of indices lands).
            w0 = cell // 16
            sidx_t = cpool.tile([128, edge_n // 16], mybir.dt.int16, tag="sidx")
            nc.sync.dma_start(out=sidx_t[:, 0:w0], in_=sidx[:, 0:w0])
            didx_t = cpool.tile([128, edge_n // 16], mybir.dt.int16, tag="didx")
            nc.sync.dma_start(out=didx_t[:, 0:w0], in_=didx[:, 0:w0])
            nc.sync.dma_start(out=sidx_t[:, w0:], in_=sidx[:, w0:])
            nc.sync.dma_start(out=didx_t[:, w0:], in_=didx[:, w0:])

            wcell = cell // 16        # idx cols per cell
            wch = CHUNK // 16         # idx cols per chunk
            jch = CHUNK // 128        # 128-edge groups per chunk
            for c in range(16):
                r, s = c // 4, c % 4
                # xr[p, h, j, f]: edge-major product, chunk ch fills j-slice
                xr_t = xrpool.tile([128, 2, nchunk, 128], f16, tag="xr")
                for ch in range(gch):
                    i0 = c * wcell + ch * wch
                    zs_t = gpool.tile([128, jch, ROW_ELEMS], f16, tag="zs")
                    nc.gpsimd.dma_gather(
                        zs_t[:], zz[:, r * ROW_ELEMS:(r + 1) * ROW_ELEMS],
                        sidx_t[:, i0:i0 + wch],
                        CHUNK, CHUNK, ROW_ELEMS,
                        elem_step=GROUP * ROW_ELEMS, transpose=False,
                        single_packet=False)
                    zd_t = gpool.tile([128, jch, ROW_ELEMS], f16, tag="zd")
                    nc.gpsimd.dma_gather(
                        zd_t[:], zz[:, s * ROW_ELEMS:(s + 1) * ROW_ELEMS],
                        didx_t[:, i0:i0 + wch],
                        CHUNK, CHUNK, ROW_ELEMS,
                        elem_step=GROUP * ROW_ELEMS, transpose=False,
                        single_packet=False)
                    j0 = ch * jch
                    nc.vector.tensor_mul(
                        out=xr_t[:, :, j0:j0 + jch, :].rearrange(
                            "p h j f -> p j h f"),
                        in0=zs_t[:].rearrange("p j (h f) -> p j h f", h=2),
                        in1=zd_t[:].rearrange("p j (h f) -> p j h f", h=2))
                # xt[f, hc, j, l] = x[edge 128j+l, feat 128hc+f]
                xt_t = xtpool.tile([128, 2, nchunk, 128], f16, tag="xt")
                nc.sync.dma_start(
                    out=xt_t[:].rearrange("p h j f -> p (h j) f"),
                    in_=xr_t[:].rearrange("p h j f -> p (h j f)"),
                    transpose=True)

                out_sb = opool.tile([1, cell], f32, tag="out")
                col = 0
                for si, w in enumerate(subs):
                    st = slice(col, col + w)
                    j0, j1 = 4 * si, 4 * si + w // 128
                    hs = []
                    for hc in range(2):
                        hp = pph.tile([128, 512], f32, tag=f"h{hc}",
                                      space="PSUM")
                        nc.tensor.matmul(
                            hp[:, :w], lhsT=w1_t[0][hc][:],
                            rhs=xt_t[:, 0, j0:j1, :].rearrange(
                                "p j f -> p (j f)"),
                            start=True, stop=False)
                        nc.tensor.matmul(
                            hp[:, :w], lhsT=w1_t[1][hc][:],
                            rhs=xt_t[:, 1, j0:j1, :].rearrange(
                                "p j f -> p (j f)"),
                            start=False, stop=True)
                        hsb = hpool.tile([128, 512], f16, tag=f"hs{hc}")
                        nc.scalar.activation(
                            hsb[:, :w], hp[:, :w],
                            mybir.ActivationFunctionType.Relu, bias=b1_t[hc][:])
                        hs.append(hsb)
                    ap = ppa.tile([1, 512], f32, tag="a", space="PSUM")
                    nc.tensor.matmul(ap[:, :w], lhsT=w2_t[0][:],
                                     rhs=hs[0][:, :w], start=True, stop=False)
                    nc.tensor.matmul(ap[:, :w], lhsT=w2_t[1][:],
                                     rhs=hs[1][:, :w], start=False, stop=True)
                    nc.scalar.activation(
                        out_sb[:, st], ap[:, :w],
                        mybir.ActivationFunctionType.Sigmoid, bias=b2_t[:])
                    col += w
                nc.sync.dma_start(out=out[c:c + 1, :], in_=out_sb[:])

    from concourse.tile_scheduler import PROC_NAME_TO_IDX
    dmasw0 = PROC_NAME_TO_IDX["DMASW0"]
    for blk in nc.m.functions[0].blocks:
        for inst in blk.instructions:
            if isinstance(inst, mybir.InstDMAGatherAnt):
                lane = inst.bass_scheduled_proc - dmasw0
                assert 0 <= lane < 8, lane
                inst.queue_num = lane % 4
    nc.compile()
    return nc


def kernel(z, z2, edge, W1, b1, W2, b2):
    from concourse.bass_utils import run_bass_kernel_spmd

    n_nodes, d = z.shape
    e_total = edge.shape[1]
    assert d == D

    src = np.asarray(edge[0], dtype=np.int64)
    dst = np.asarray(edge[1], dtype=np.int64)

    n_pad = _round_up(n_nodes, GROUP)
    zz = np.zeros((n_pad, ROW_ELEMS), dtype=np.float16)
    zz[:n_nodes, :D] = z.astype(np.float16)
    zz[:n_nodes, D:] = z2.astype(np.float16)
    zz_g = zz.reshape(n_pad // GROUP, GROUP * ROW_ELEMS)

    cls = (src % GROUP) * 4 + (dst % GROUP)
    order = np.argsort(cls, kind="stable")
    bounds = np.searchsorted(cls[order], np.arange(17))
    splits = {}
    max_cnt = 0
    for ci in range(16):
        ids = order[bounds[ci]:bounds[ci + 1]]
        parts = np.array_split(ids, N_CORES)
        for core in range(N_CORES):
            splits[(core, ci)] = parts[core]
            max_cnt = max(max_cnt, len(parts[core]))
    cell = max(_round_up(max_cnt, CHUNK), CHUNK)
    edge_n = 16 * cell

    in_maps = []
    perms = []
    for core in range(N_CORES):
        s_loc = np.zeros(edge_n, dtype=np.int64)
        d_loc = np.zeros(edge_n, dtype=np.int64)
        perm = np.full(edge_n, -1, dtype=np.int64)
        for ci in range(16):
            ids = splits[(core, ci)]
            o = ci * cell
            n = len(ids)
            perm[o:o + n] = ids
            s_loc[o:o + n] = src[ids] // GROUP
            d_loc[o:o + n] = dst[ids] // GROUP
        s16 = s_loc.astype(np.int16)
        d16 = d_loc.astype(np.int16)
        sw = np.concatenate(
            [s16[c * cell:(c + 1) * cell].reshape(cell // 16, 16).T
             for c in range(16)], axis=1)
        dw = np.concatenate(
            [d16[c * cell:(c + 1) * cell].reshape(cell // 16, 16).T
             for c in range(16)], axis=1)
        in_maps.append({
            "zz": zz_g,
            "sidx": np.tile(sw, (8, 1)),
            "didx": np.tile(dw, (8, 1)),
            "w1t": np.ascontiguousarray(W1.T).astype(np.float16),
            "b1": np.asarray(b1, dtype=np.float32).reshape(H, 1),
            "w2t": np.ascontiguousarray(np.asarray(W2).T).astype(np.float16),
            "b2": np.asarray(b2, dtype=np.float32).reshape(1, 1),
        })
        perms.append(perm)

    key = (cell, zz_g.shape[0])
    if key not in _prog_cache:
        _prog_cache[key] = _build_program(cell, zz_g.shape[0])
    nc = _prog_cache[key]

    res = run_bass_kernel_spmd(nc, in_maps, core_ids=list(range(N_CORES)))

    out_full = np.empty((e_total,), dtype=np.float32)
    for core in range(N_CORES):
        vals = np.asarray(res.results[core]["out"], dtype=np.float32).ravel()
        p = perms[core]
        m = p >= 0
        out_full[p[m]] = vals[m]
    return out_full.reshape(e_total, 1)
